# revision 24
# baseline (speedup 1.0000x reference)
"""Dynamic lightweight convolution TRN2 kernel.

out[b,l,d] = (1/K) * sum_k softmax_k(x[b,l+K-1,:] @ W + bias)[k, d%H] * x[b,l+k,d]

B=8, S=2048, D=1024, K=7, H=16, L=S-K+1=2042.
Sharding: data-parallel over batch, one batch element per NeuronCore (8 cores).

Per-core plan (channels on partitions; sequence on the free axis so the K=7
window shifts are free-axis offsets):
  1. DMA x in 8 blocks of 256 rows; PE-transpose 128x128 tiles; ScalarE
     copies PSUM->SBUF casting to bf16 -> xtb[d, s].
  2. logits = W^T @ xT on PE (bf16, fp32 PSUM accumulation over 8 d-chunks).
  3. E = exp(logits + bias) (ScalarE); a [112,112] selector matmul computes
     K*sum_k E; Rinv = 1/that (DVE); en = E * Rinv (DVE).
  4. m[p, k, l] = en[16k + p%16, l+6] via [112,128] 0/1 selector matmuls (PE)
     + ScalarE casts to bf16. One m tile serves all 8 d-chunks (weight for
     channel d = 128c + p is row p%16 = d%16).
  5. conv products p_k = m_k * xtb_{+k}: DVE tensor_mul with the m operand
     broadcast (stride 0) across d-chunks, one instruction per (tap, piece);
     a slice of taps for chunk 7 runs on GPSIMD(Pool) to offload DVE.
  6. tap reduction sum_k p_k, routed per (piece, chunk):
     - chunks 0..5: PE as 7 accumulating identity matmuls into PSUM (f32),
       then ScalarE PSUM->SBUF bf16 copy (engine with slack);
     - chunks 6..7: add tree on Pool (early pieces) / DVE (late pieces).
  7. out is written TRANSPOSED to DRAM as [128, C, L] bf16 (d-major); the
     host transposes to [L, D] and upcasts to f32 (numerically identical to
     a device-side bf16 store + cast). This removes the output PE transpose
     pass entirely.
Engine balance targets (cost model): DVE ~muls, PE ~fixed+tap-sums,
Pool ~c6/c7 sums + some c7 muls, ACT ~copies, DMA ~39us.
"""

import numpy as np
import ml_dtypes
from contextlib import ExitStack

import concourse.bacc as bacc
import concourse.tile as tile
from concourse import mybir
from concourse import bass_utils

K = 7
H = 16
B, S, D = 8, 2048, 1024
L = S - K + 1  # 2042
C = D // 128  # 8 d-chunks
NSB = 4  # 512-col s-blocks
SB = S // NSB  # 512
KH = K * H  # 112

F32 = mybir.dt.float32
BF16 = mybir.dt.bfloat16

# Conv l-block boundaries, aligned so conv block i only reads xtb columns
# and en columns (l+6) below a 256-col front boundary. First two blocks are
# small (250/256) to shorten the pipeline ramp; the rest are 512 wide.
CBL = [0, 250, 506, 1018, 1530, 2042]
NI = len(CBL) - 1
# front 256-col t-blocks that must be complete before mrep/conv of block i
T_NEED = [1, 2, 4, 6, 8]

# Conv routing by global piece index p (blocks i0,i1 have 1 piece; i2..i4
# have 2): p = 0..7.
POOL_MUL_C7_P = (0, 1, 2, 3, 4, 5)  # c7 mul taps on Pool

# byte offsets (per partition) inside the packed constants blob
_OFF_BIAS = 0      # [112, 1] f32
_OFF_IDENT = 4     # [128, 128] f32
_OFF_IDENTB = 516  # [128, 128] bf16
_OFF_SELSUM = 772  # [112, 112] bf16
_OFF_SELK = 996    # [112, 896] bf16
_OFF_WT = 2788     # [128, 8, 112] bf16
_CONST_BYTES = 4580  # 1145 f32 columns


def _host_constants(W, b):
    """Pack bias/ident/identb/selsum/selk/W into one [128, 1145] f32 blob."""
    buf = np.zeros((128, _CONST_BYTES), np.uint8)

    def put(off, arr):
        by = np.ascontiguousarray(arr).view(np.uint8).reshape(arr.shape[0], -1)
        buf[: arr.shape[0], off : off + by.shape[1]] = by

    put(_OFF_BIAS, np.asarray(b, np.float32).reshape(KH, 1))
    put(_OFF_IDENT, np.eye(128, dtype=np.float32))
    put(_OFF_IDENTB, np.eye(128).astype(ml_dtypes.bfloat16))
    h = np.arange(KH) % H
    selsum = ((h[:, None] == h[None, :]) * float(K)).astype(ml_dtypes.bfloat16)
    put(_OFF_SELSUM, selsum)
    selk = np.zeros((KH, K * 128), dtype=ml_dtypes.bfloat16)
    for k in range(K):
        for p in range(128):
            selk[16 * k + p % 16, k * 128 + p] = 1.0
    put(_OFF_SELK, selk)
    # W [D, KH] -> [128, C, KH] chunks (d = c*128 + p)
    wt = np.asarray(W, np.float32).astype(ml_dtypes.bfloat16)
    wt = wt.reshape(C, 128, KH).transpose(1, 0, 2).reshape(128, C * KH)
    put(_OFF_WT, np.ascontiguousarray(wt))
    return buf.view(np.float32)


PHASE_LOG = []


def build_program():
    PHASE_LOG.clear()
    nc = bacc.Bacc(
        "TRN2", target_bir_lowering=False, debug=False, enable_asserts=True
    )

    x_d = nc.dram_tensor("x", [S, D], F32, kind="ExternalInput").ap()
    consts_d = nc.dram_tensor(
        "consts", [128, _CONST_BYTES // 4], F32, kind="ExternalInput"
    ).ap()
    # transposed output: outT[p, c, l] = out[l, 128c + p], bf16
    out_d = nc.dram_tensor("out", [128, C, L], BF16, kind="ExternalOutput").ap()

    with tile.TileContext(nc) as tc, ExitStack() as ctx:
        singles = ctx.enter_context(tc.tile_pool(name="singles", bufs=1))
        xn_pool = ctx.enter_context(tc.tile_pool(name="xn", bufs=4))
        m_pool = ctx.enter_context(tc.tile_pool(name="mw", bufs=3))
        prod_pool = ctx.enter_context(tc.tile_pool(name="prod", bufs=5))
        tmp_pool = ctx.enter_context(tc.tile_pool(name="tmp", bufs=6))
        acc_pool = ctx.enter_context(tc.tile_pool(name="accb", bufs=3))
        st_pool = ctx.enter_context(tc.tile_pool(name="stg", bufs=4))

        p_tp = ctx.enter_context(tc.tile_pool(name="ptp", bufs=2, space="PSUM"))
        p_log = ctx.enter_context(tc.tile_pool(name="plog", bufs=1, space="PSUM"))
        p_sum = ctx.enter_context(tc.tile_pool(name="psumk", bufs=1, space="PSUM"))
        p_mk = ctx.enter_context(tc.tile_pool(name="pmk", bufs=2, space="PSUM"))
        p_conv = ctx.enter_context(tc.tile_pool(name="pconv", bufs=2, space="PSUM"))

        # ---- constants: split DMA (identities first, rest after load 0)
        # so the first PE transposes aren't gated on the full 586KB blob ----
        cblob = singles.tile([128, _CONST_BYTES // 4], F32)
        _C1 = _OFF_SELSUM // 4  # bias+ident+identb
        nc.sync.dma_start(out=cblob[:, :_C1], in_=consts_d[:, :_C1])

        def load_rest_consts():
            nc.sync.dma_start(out=cblob[:, _C1:], in_=consts_d[:, _C1:])

        cbytes = cblob.bitcast(mybir.dt.uint8)

        def cview(off, nbytes, dt, rows=128):
            return cbytes[:rows, off : off + nbytes].bitcast(dt)

        bias_t = cview(_OFF_BIAS, 4, F32, rows=KH)
        ident_t = cview(_OFF_IDENT, 512, F32)
        identb_t = cview(_OFF_IDENTB, 256, BF16)
        selsum_t = cview(_OFF_SELSUM, 224, BF16, rows=KH)
        selk_t = cview(_OFF_SELK, 1792, BF16, rows=KH).rearrange(
            "c (k p) -> c k p", k=K
        )
        wt = cview(_OFF_WT, 1792, BF16).rearrange("p (c n) -> p c n", c=C)

        # GPSIMD ucode warmup + ACT exp table load, early so they overlap DMA
        warm = singles.tile([1, 8], BF16)
        nc.gpsimd.tensor_mul(warm, identb_t[:1, :8], identb_t[:1, :8])
        warma = singles.tile([1, 8], F32)
        nc.scalar.activation(
            warma, ident_t[:1, :8], mybir.ActivationFunctionType.Exp
        )

        # ---- persistent tensors ----
        xtb = singles.tile([128, C, S], BF16)  # x^T bf16
        e_full = singles.tile([KH, S], BF16)  # exp(logits + b)
        ssum_sb = singles.tile([KH, S], F32)  # K * sum_k E (staged from PSUM)
        rinv = singles.tile([KH, S], F32)  # 1 / (K * sum_k E)
        en = singles.tile([KH, S], BF16)  # normalized kernel weights

        xn_tiles = {}

        def load(t):  # 256-row block
            xn = xn_pool.tile([128, 2, D], F32, tag="xn")
            xin = x_d[256 * t : 256 * (t + 1), :].rearrange(
                "(t p) d -> p t d", p=128
            )
            if t <= 1:
                # split so the first transposes start after a half-load
                nc.sync.dma_start(out=xn[:, 0:1, :], in_=xin[:, 0:1, :])
                nc.sync.dma_start(out=xn[:, 1:2, :], in_=xin[:, 1:2, :])
            else:
                nc.sync.dma_start(out=xn, in_=xin)
            xn_tiles[t] = xn

        def front(t):
            """Transpose 256-col t-block to [d, s]; logits matmul; exp;
            softmax denominators + normalized weights."""
            s0 = 256 * t
            for c in range(C):
                ptp = p_tp.tile([128, 256], F32, tag="ptp")
                for tt in range(2):
                    nc.tensor.transpose(
                        ptp[:, 128 * tt : 128 * (tt + 1)],
                        xn_tiles[t][:, tt, 128 * c : 128 * (c + 1)],
                        ident_t,
                    )
                nc.scalar.copy(xtb[:, c, s0 : s0 + 256], ptp)
            plog = p_log.tile([KH, 256], F32, tag="plog")
            for c in range(C):
                nc.tensor.matmul(
                    plog,
                    wt[:, c, :],
                    xtb[:, c, s0 : s0 + 256],
                    start=(c == 0),
                    stop=(c == C - 1),
                )
            sl = slice(s0, s0 + 256)
            nc.scalar.activation(
                e_full[:, sl],
                plog,
                mybir.ActivationFunctionType.Exp,
                bias=bias_t,
                scale=1.0,
            )
            psum = p_sum.tile([KH, 256], F32, tag="psumk")
            nc.tensor.matmul(psum, selsum_t, e_full[:, sl], start=True, stop=True)
            # stage the denominators to SBUF immediately (frees the PSUM
            # bank; lets the DVE recip run much later without holding it)
            nc.scalar.copy(ssum_sb[:, sl], psum)

        def denom(t):
            """DVE part of the softmax denominators (emitted separately so
            conv muls are not stuck behind it in DVE's in-order stream)."""
            sl = slice(256 * t, 256 * (t + 1))
            nc.vector.reciprocal(rinv[:, sl], ssum_sb[:, sl])
            nc.vector.tensor_mul(en[:, sl], e_full[:, sl], rinv[:, sl])

        m_tiles = {}

        def mrep(i):
            """m_i[p, k, l-CBL[i]] = en[16k + p%16, l + 6] for conv block i."""
            nl = CBL[i + 1] - CBL[i]
            mj = m_pool.tile([128, K, SB], BF16, tag="mw")
            for k in range(K):
                pmk = p_mk.tile([128, SB], F32, tag="pmk")
                nc.tensor.matmul(
                    pmk[:, :nl],
                    selk_t[:, k, :],
                    en[:, CBL[i] + K - 1 : CBL[i] + K - 1 + nl],
                    start=True,
                    stop=True,
                )
                nc.scalar.copy(mj[:, k, :nl], pmk[:, :nl])
            m_tiles[i] = mj

        piece_of = []  # global piece index per (i, half)
        _pc = [0]
        for _i in range(NI):
            _n = 1 if CBL[_i + 1] - CBL[_i] <= 256 else 2
            piece_of.append([_pc[0] + h for h in range(_n)])
            _pc[0] += _n

        def conv_pieces(i):
            nli = CBL[i + 1] - CBL[i]
            if nli <= 256:
                return [(0, nli)]
            return [(0, nli - 256), (nli - 256, 256)]

        halves_of = {}

        def _tree6(eng, srcs, dst, nl):
            """6-op add tree over 7 product slices (each [128, n, nl])."""
            ts = [
                tmp_pool.tile([128, 2, 256], BF16, tag="tmp", name=f"ts{q}")
                for q in range(5)
            ]
            eng.tensor_add(ts[0][:, :, :nl], srcs[0], srcs[1])
            eng.tensor_add(ts[1][:, :, :nl], srcs[2], srcs[3])
            eng.tensor_add(ts[2][:, :, :nl], srcs[4], srcs[5])
            eng.tensor_add(ts[3][:, :, :nl], ts[0][:, :, :nl], ts[1][:, :, :nl])
            eng.tensor_add(ts[4][:, :, :nl], ts[2][:, :, :nl], srcs[6])
            eng.tensor_add(dst, ts[3][:, :, :nl], ts[4][:, :, :nl])

        def conv_muls(i):
            """Products for conv l-block i (DVE/Pool) + non-PE tap-sums.
            Products live in two 4-chunk tiles (A: c0..3, B: c4..7) so PE
            tap-sums can start after half the muls and slots recycle finer."""
            mj = m_tiles[i]
            last = i == NI - 1
            halves = halves_of[i] = []
            for half, (off, nl) in enumerate(conv_pieces(i)):
                l0 = CBL[i] + off
                p = piece_of[i][half]
                Pa = prod_pool.tile([128, K, 4, 256], BF16, tag="prod", name="Pa")
                Pb = prod_pool.tile([128, K, 4, 256], BF16, tag="prod", name="Pb")
                halves.append((off, nl, Pa, Pb))
                pool_c7 = p in POOL_MUL_C7_P
                # Pool muls first so the slow engine starts early
                if pool_c7:
                    for k in range(K):
                        nc.gpsimd.tensor_mul(
                            Pb[:, k, 3, :nl],
                            mj[:, k, off : off + nl],
                            xtb[:, 7, l0 + k : l0 + k + nl],
                        )
                for P, c0, c1 in ((Pa, 0, 4), (Pb, 4, 7 if pool_c7 else 8)):
                    for k in range(K):
                        mb = mj[:, k : k + 1, off : off + nl].broadcast_to(
                            (128, c1 - c0, nl)
                        )
                        nc.vector.tensor_mul(
                            P[:, k, 0 : c1 - c0, :nl],
                            mb,
                            xtb[:, c0:c1, l0 + k : l0 + k + nl],
                        )
                if last:
                    # c0/c1 tap-sum on DVE right after its own muls
                    acc = acc_pool.tile([128, 2, 256], BF16, tag="accb")
                    _tree6(
                        nc.vector,
                        [Pa[:, k, 0:2, :nl] for k in range(K)],
                        acc[:, :, :nl],
                        nl,
                    )
                    nc.sync.dma_start(
                        out=out_d[:, 0:2, l0 : l0 + nl], in_=acc[:, :, :nl]
                    )
                if not last:
                    # c6/c7 tap-sum tree on Pool
                    acc2 = acc_pool.tile([128, 2, 256], BF16, tag="accb")
                    _tree6(
                        nc.gpsimd,
                        [Pb[:, k, 2:4, :nl] for k in range(K)],
                        acc2[:, :, :nl],
                        nl,
                    )
                    nc.sync.dma_start(
                        out=out_d[:, 6:8, l0 : l0 + nl], in_=acc2[:, :, :nl]
                    )

        def conv_trees(i):
            """PE tap-sums: all pieces into one PSUM tile per chunk, drained
            per chunk (ACT bf16 copy + DMA) so <=bufs tiles live."""
            nli = CBL[i + 1] - CBL[i]
            last = i == NI - 1
            halves = halves_of.pop(i)
            cstart, cend = (0, 6) if not last else (2, 8)
            for c in range(cstart, cend):
                pc = p_conv.tile([128, SB], F32, tag="pconv")
                for off, nl, Pa, Pb in halves:
                    P, ci = (Pa, c) if c < 4 else (Pb, c - 4)
                    for k in range(K):
                        nc.tensor.matmul(
                            pc[:, off : off + nl],
                            identb_t,
                            P[:, k, ci, :nl],
                            start=(k == 0),
                            stop=(k == K - 1),
                        )
                st = st_pool.tile([128, SB], BF16, tag="stg")
                nc.scalar.copy(st[:, :nli], pc[:, :nli])
                nc.sync.dma_start(
                    out=out_d[:, c, CBL[i] : CBL[i + 1]], in_=st[:, :nli]
                )

        # ---- pipelined emission ----
        # front/mrep prep for block i+1 is emitted before conv(i) so
        # engines with in-order streams don't stall at block boundaries.
        def mark(label):
            PHASE_LOG.append((label, nc.next_id()))

        # Lookahead-2 pipeline: fronts/denoms/mrep for block i+2 are emitted
        # around conv(i), so the mrep PE->PSUM->ACT latency never gates the
        # conv muls, and DVE's recips run when their selsums are long done.
        load(0)
        load_rest_consts()
        for t in range(1, 8):
            load(t)
        mark("front0")
        front(0)
        denom(0)
        mark("mrep0")
        mrep(0)
        mark("front1")
        front(1)
        denom(1)
        mark("mrep1")
        mrep(1)
        t_done = 2
        d_done = 2  # denoms/mreps emitted one iteration after their fronts
        pend = []
        for i in range(NI):
            mark(f"conv{i}m")
            conv_muls(i)
            mark(f"conv{i}t")
            conv_trees(i)
            if pend:
                mi, ts = pend.pop(0)
                mark(f"mrep{mi}")
                for t in ts:
                    denom(t)
                mrep(mi)
            if i + 2 < NI:
                new_ts = []
                while t_done < T_NEED[i + 2]:
                    mark(f"front{t_done}")
                    front(t_done)
                    new_ts.append(t_done)
                    t_done += 1
                pend.append((i + 2, new_ts))
        mark("end")

    nc.compile()
    return nc


_CACHE = {}


def _get_program():
    if "nc" not in _CACHE:
        _CACHE["nc"] = build_program()
    return _CACHE["nc"]


def kernel(x, W, b):
    x = np.asarray(x, dtype=np.float32)
    assert x.shape == (B, S, D), x.shape

    nc = _get_program()
    consts = _host_constants(W, b)
    in_maps = []
    for core in range(B):
        in_maps.append(
            {
                "x": np.ascontiguousarray(x[core]),
                "consts": consts,
            }
        )
    res = bass_utils.run_bass_kernel_spmd(nc, in_maps, core_ids=list(range(B)))
    out = np.empty((B, L, D), dtype=np.float32)
    for core in range(B):
        ot = np.asarray(res.results[core]["out"])  # [128, C, L] bf16
        # d = 128*c + p  ->  [C, 128, L] -> [D, L] -> [L, D]
        out[core] = ot.transpose(1, 0, 2).reshape(D, L).T.astype(np.float32)
    return out


# revision 25
# speedup vs baseline: 1.0837x; 1.0837x over previous
"""Dynamic lightweight convolution TRN2 kernel.

out[b,l,d] = (1/K) * sum_k softmax_k(x[b,l+K-1,:] @ W + bias)[k, d%H] * x[b,l+k,d]

B=8, S=2048, D=1024, K=7, H=16, L=S-K+1=2042.
Sharding: data-parallel over batch, one batch element per NeuronCore (8 cores).

Per-core plan (channels on partitions; sequence on the free axis so the K=7
window shifts are free-axis offsets):
  1. DMA x in 8 blocks of 256 rows; PE-transpose 128x128 tiles; ScalarE
     copies PSUM->SBUF casting to bf16 -> xtb[d, s].
  2. logits = W^T @ xT on PE (bf16, fp32 PSUM accumulation over 8 d-chunks).
  3. E = exp(logits + bias) (ScalarE); a [112,112] selector matmul computes
     K*sum_k E; Rinv = 1/that (DVE); en = E * Rinv (DVE).
  4. m[p, k, l] = en[16k + p%16, l+6] via [112,128] 0/1 selector matmuls (PE)
     + ScalarE casts to bf16. One m tile serves all 8 d-chunks (weight for
     channel d = 128c + p is row p%16 = d%16).
  5. conv products p_k = m_k * xtb_{+k}: DVE tensor_mul with the m operand
     broadcast (stride 0) across d-chunks, one instruction per (tap, piece);
     a slice of taps for chunk 7 runs on GPSIMD(Pool) to offload DVE.
  6. tap reduction sum_k p_k, routed per (piece, chunk):
     - chunks 0..5: PE as 7 accumulating identity matmuls into PSUM (f32),
       then ScalarE PSUM->SBUF bf16 copy (engine with slack);
     - chunks 6..7: add tree on Pool (early pieces) / DVE (late pieces).
  7. out is written TRANSPOSED to DRAM as [128, C, L] bf16 (d-major); the
     host transposes to [L, D] and upcasts to f32 (numerically identical to
     a device-side bf16 store + cast). This removes the output PE transpose
     pass entirely.
Engine balance targets (cost model): DVE ~muls, PE ~fixed+tap-sums,
Pool ~c6/c7 sums + some c7 muls, ACT ~copies, DMA ~39us.
"""

import numpy as np
import ml_dtypes
from contextlib import ExitStack

import concourse.bacc as bacc
import concourse.tile as tile
from concourse import mybir
from concourse import bass_utils

K = 7
H = 16
B, S, D = 8, 2048, 1024
L = S - K + 1  # 2042
C = D // 128  # 8 d-chunks
NSB = 4  # 512-col s-blocks
SB = S // NSB  # 512
KH = K * H  # 112

F32 = mybir.dt.float32
BF16 = mybir.dt.bfloat16

# Conv l-block boundaries, aligned so conv block i only reads xtb columns
# and en columns (l+6) below a 256-col front boundary. First two blocks are
# small (250/256) to shorten the pipeline ramp; the rest are 512 wide.
CBL = [0, 250, 506, 1018, 1530, 2042]
NI = len(CBL) - 1
# front 256-col t-blocks that must be complete before mrep/conv of block i
T_NEED = [1, 2, 4, 6, 8]

# Conv routing by global piece index p (blocks i0,i1 have 1 piece; i2..i4
# have 2): p = 0..7.
POOL_MUL_C7_P = (0, 1, 2, 3, 4, 5)  # c7 mul taps on Pool

# byte offsets (per partition) inside the packed constants blob
_OFF_BIAS = 0      # [112, 1] f32
_OFF_IDENT = 4     # [128, 128] f32
_OFF_IDENTB = 516  # [128, 128] bf16
_OFF_SELSUM = 772  # [112, 112] bf16
_OFF_SELK = 996    # [112, 896] bf16
_OFF_WT = 2788     # [128, 8, 112] bf16
_CONST_BYTES = 4580  # 1145 f32 columns


def _host_constants(W, b):
    """Pack bias/ident/identb/selsum/selk/W into one [128, 1145] f32 blob."""
    buf = np.zeros((128, _CONST_BYTES), np.uint8)

    def put(off, arr):
        by = np.ascontiguousarray(arr).view(np.uint8).reshape(arr.shape[0], -1)
        buf[: arr.shape[0], off : off + by.shape[1]] = by

    put(_OFF_BIAS, np.asarray(b, np.float32).reshape(KH, 1))
    put(_OFF_IDENT, np.eye(128, dtype=np.float32))
    put(_OFF_IDENTB, np.eye(128).astype(ml_dtypes.bfloat16))
    h = np.arange(KH) % H
    selsum = ((h[:, None] == h[None, :]) * float(K)).astype(ml_dtypes.bfloat16)
    put(_OFF_SELSUM, selsum)
    selk = np.zeros((KH, K * 128), dtype=ml_dtypes.bfloat16)
    for k in range(K):
        for p in range(128):
            selk[16 * k + p % 16, k * 128 + p] = 1.0
    put(_OFF_SELK, selk)
    # W [D, KH] -> [128, C, KH] chunks (d = c*128 + p)
    wt = np.asarray(W, np.float32).astype(ml_dtypes.bfloat16)
    wt = wt.reshape(C, 128, KH).transpose(1, 0, 2).reshape(128, C * KH)
    put(_OFF_WT, np.ascontiguousarray(wt))
    return buf.view(np.float32)


PHASE_LOG = []


def build_program():
    PHASE_LOG.clear()
    nc = bacc.Bacc(
        "TRN2", target_bir_lowering=False, debug=False, enable_asserts=True
    )

    x_d = nc.dram_tensor("x", [S, D], F32, kind="ExternalInput").ap()
    consts_d = nc.dram_tensor(
        "consts", [128, _CONST_BYTES // 4], F32, kind="ExternalInput"
    ).ap()
    # transposed output: outT[p, c, l] = out[l, 128c + p], bf16
    out_d = nc.dram_tensor("out", [128, C, L], BF16, kind="ExternalOutput").ap()

    with tile.TileContext(nc) as tc, ExitStack() as ctx:
        singles = ctx.enter_context(tc.tile_pool(name="singles", bufs=1))
        xn_pool = ctx.enter_context(tc.tile_pool(name="xn", bufs=4))
        m_pool = ctx.enter_context(tc.tile_pool(name="mw", bufs=3))
        prod_pool = ctx.enter_context(tc.tile_pool(name="prod", bufs=5))
        tmp_pool = ctx.enter_context(tc.tile_pool(name="tmp", bufs=6))
        acc_pool = ctx.enter_context(tc.tile_pool(name="accb", bufs=3))
        st_pool = ctx.enter_context(tc.tile_pool(name="stg", bufs=4))

        p_tp = ctx.enter_context(tc.tile_pool(name="ptp", bufs=2, space="PSUM"))
        p_log = ctx.enter_context(tc.tile_pool(name="plog", bufs=1, space="PSUM"))
        p_sum = ctx.enter_context(tc.tile_pool(name="psumk", bufs=1, space="PSUM"))
        p_mk = ctx.enter_context(tc.tile_pool(name="pmk", bufs=2, space="PSUM"))
        p_conv = ctx.enter_context(tc.tile_pool(name="pconv", bufs=2, space="PSUM"))

        # ---- constants: split DMA (identities first, rest after load 0)
        # so the first PE transposes aren't gated on the full 586KB blob ----
        cblob = singles.tile([128, _CONST_BYTES // 4], F32)
        _C1 = _OFF_SELSUM // 4  # bias+ident+identb
        nc.sync.dma_start(out=cblob[:, :_C1], in_=consts_d[:, :_C1])

        def load_rest_consts():
            nc.sync.dma_start(out=cblob[:, _C1:], in_=consts_d[:, _C1:])

        cbytes = cblob.bitcast(mybir.dt.uint8)

        def cview(off, nbytes, dt, rows=128):
            return cbytes[:rows, off : off + nbytes].bitcast(dt)

        bias_t = cview(_OFF_BIAS, 4, F32, rows=KH)
        ident_t = cview(_OFF_IDENT, 512, F32)
        identb_t = cview(_OFF_IDENTB, 256, BF16)
        selsum_t = cview(_OFF_SELSUM, 224, BF16, rows=KH)
        selk_t = cview(_OFF_SELK, 1792, BF16, rows=KH).rearrange(
            "c (k p) -> c k p", k=K
        )
        wt = cview(_OFF_WT, 1792, BF16).rearrange("p (c n) -> p c n", c=C)

        # GPSIMD ucode warmup + ACT exp table load, early so they overlap DMA
        warm = singles.tile([1, 8], BF16)
        nc.gpsimd.tensor_mul(warm, identb_t[:1, :8], identb_t[:1, :8])
        warma = singles.tile([1, 8], F32)
        nc.scalar.activation(
            warma, ident_t[:1, :8], mybir.ActivationFunctionType.Exp
        )

        # ---- persistent tensors ----
        xtb = singles.tile([128, C, S], BF16)  # x^T bf16
        e_full = singles.tile([KH, S], BF16)  # exp(logits + b)
        ssum_sb = singles.tile([KH, S], F32)  # K * sum_k E (staged from PSUM)
        rinv = singles.tile([KH, S], F32)  # 1 / (K * sum_k E)
        en = singles.tile([KH, S], BF16)  # normalized kernel weights

        xn_tiles = {}

        def load(t):  # 256-row block
            xn = xn_pool.tile([128, 2, D], F32, tag="xn")
            xin = x_d[256 * t : 256 * (t + 1), :].rearrange(
                "(t p) d -> p t d", p=128
            )
            if t <= 1:
                # split so the first transposes start after a half-load
                nc.sync.dma_start(out=xn[:, 0:1, :], in_=xin[:, 0:1, :])
                nc.sync.dma_start(out=xn[:, 1:2, :], in_=xin[:, 1:2, :])
            else:
                nc.sync.dma_start(out=xn, in_=xin)
            xn_tiles[t] = xn

        def front(t):
            """Transpose 256-col t-block to [d, s]; logits matmul; exp;
            softmax denominators + normalized weights."""
            s0 = 256 * t
            for c in range(C):
                ptp = p_tp.tile([128, 256], F32, tag="ptp")
                for tt in range(2):
                    nc.tensor.transpose(
                        ptp[:, 128 * tt : 128 * (tt + 1)],
                        xn_tiles[t][:, tt, 128 * c : 128 * (c + 1)],
                        ident_t,
                    )
                nc.scalar.copy(xtb[:, c, s0 : s0 + 256], ptp)
            plog = p_log.tile([KH, 256], F32, tag="plog")
            for c in range(C):
                nc.tensor.matmul(
                    plog,
                    wt[:, c, :],
                    xtb[:, c, s0 : s0 + 256],
                    start=(c == 0),
                    stop=(c == C - 1),
                )
            sl = slice(s0, s0 + 256)
            nc.scalar.activation(
                e_full[:, sl],
                plog,
                mybir.ActivationFunctionType.Exp,
                bias=bias_t,
                scale=1.0,
            )
            psum = p_sum.tile([KH, 256], F32, tag="psumk")
            nc.tensor.matmul(psum, selsum_t, e_full[:, sl], start=True, stop=True)
            # stage the denominators to SBUF immediately (frees the PSUM
            # bank; lets the DVE recip run much later without holding it)
            nc.scalar.copy(ssum_sb[:, sl], psum)

        def denom(t):
            """DVE part of the softmax denominators (emitted separately so
            conv muls are not stuck behind it in DVE's in-order stream)."""
            sl = slice(256 * t, 256 * (t + 1))
            nc.vector.reciprocal(rinv[:, sl], ssum_sb[:, sl])
            nc.vector.tensor_mul(en[:, sl], e_full[:, sl], rinv[:, sl])

        m_tiles = {}

        def mrep(i):
            """m_i[p, k, l-CBL[i]] = en[16k + p%16, l + 6] for conv block i."""
            nl = CBL[i + 1] - CBL[i]
            mj = m_pool.tile([128, K, SB], BF16, tag="mw")
            for k in range(K):
                pmk = p_mk.tile([128, SB], F32, tag="pmk")
                nc.tensor.matmul(
                    pmk[:, :nl],
                    selk_t[:, k, :],
                    en[:, CBL[i] + K - 1 : CBL[i] + K - 1 + nl],
                    start=True,
                    stop=True,
                )
                nc.scalar.copy(mj[:, k, :nl], pmk[:, :nl])
            m_tiles[i] = mj

        piece_of = []  # global piece index per (i, half)
        _pc = [0]
        for _i in range(NI):
            _n = 1 if CBL[_i + 1] - CBL[_i] <= 256 else 2
            piece_of.append([_pc[0] + h for h in range(_n)])
            _pc[0] += _n

        def conv_pieces(i):
            nli = CBL[i + 1] - CBL[i]
            if nli <= 256:
                return [(0, nli)]
            return [(0, nli - 256), (nli - 256, 256)]

        halves_of = {}

        def _tree6(eng, srcs, dst, nl):
            """6-op add tree over 7 product slices (each [128, n, nl])."""
            ts = [
                tmp_pool.tile([128, 2, 256], BF16, tag="tmp", name=f"ts{q}")
                for q in range(5)
            ]
            eng.tensor_add(ts[0][:, :, :nl], srcs[0], srcs[1])
            eng.tensor_add(ts[1][:, :, :nl], srcs[2], srcs[3])
            eng.tensor_add(ts[2][:, :, :nl], srcs[4], srcs[5])
            eng.tensor_add(ts[3][:, :, :nl], ts[0][:, :, :nl], ts[1][:, :, :nl])
            eng.tensor_add(ts[4][:, :, :nl], ts[2][:, :, :nl], srcs[6])
            eng.tensor_add(dst, ts[3][:, :, :nl], ts[4][:, :, :nl])

        def conv_muls(i):
            """Products for conv l-block i (DVE/Pool) + non-PE tap-sums.
            Products live in two 4-chunk tiles (A: c0..3, B: c4..7) so PE
            tap-sums can start after half the muls and slots recycle finer."""
            mj = m_tiles[i]
            last = i == NI - 1
            halves = halves_of[i] = []
            for half, (off, nl) in enumerate(conv_pieces(i)):
                l0 = CBL[i] + off
                p = piece_of[i][half]
                Pa = prod_pool.tile([128, K, 4, 256], BF16, tag="prod", name="Pa")
                Pb = prod_pool.tile([128, K, 4, 256], BF16, tag="prod", name="Pb")
                halves.append((off, nl, Pa, Pb))
                pool_c7 = p in POOL_MUL_C7_P
                # Pool muls first so the slow engine starts early
                if pool_c7:
                    for k in range(K):
                        nc.gpsimd.tensor_mul(
                            Pb[:, k, 3, :nl],
                            mj[:, k, off : off + nl],
                            xtb[:, 7, l0 + k : l0 + k + nl],
                        )
                for P, c0, c1 in ((Pa, 0, 4), (Pb, 4, 7 if pool_c7 else 8)):
                    for k in range(K):
                        mb = mj[:, k : k + 1, off : off + nl].broadcast_to(
                            (128, c1 - c0, nl)
                        )
                        nc.vector.tensor_mul(
                            P[:, k, 0 : c1 - c0, :nl],
                            mb,
                            xtb[:, c0:c1, l0 + k : l0 + k + nl],
                        )
                if last:
                    # c0/c1 tap-sum on DVE right after its own muls
                    acc = acc_pool.tile([128, 2, 256], BF16, tag="accb")
                    _tree6(
                        nc.vector,
                        [Pa[:, k, 0:2, :nl] for k in range(K)],
                        acc[:, :, :nl],
                        nl,
                    )
                    nc.sync.dma_start(
                        out=out_d[:, 0:2, l0 : l0 + nl], in_=acc[:, :, :nl]
                    )
                if not last:
                    # c6/c7 tap-sum tree on Pool
                    acc2 = acc_pool.tile([128, 2, 256], BF16, tag="accb")
                    _tree6(
                        nc.gpsimd,
                        [Pb[:, k, 2:4, :nl] for k in range(K)],
                        acc2[:, :, :nl],
                        nl,
                    )
                    nc.sync.dma_start(
                        out=out_d[:, 6:8, l0 : l0 + nl], in_=acc2[:, :, :nl]
                    )

        def conv_trees(i):
            """PE tap-sums: all pieces into one PSUM tile per chunk, drained
            per chunk (ACT bf16 copy + DMA) so <=bufs tiles live."""
            nli = CBL[i + 1] - CBL[i]
            last = i == NI - 1
            halves = halves_of.pop(i)
            cstart, cend = (0, 6) if not last else (2, 8)
            for c in range(cstart, cend):
                pc = p_conv.tile([128, SB], F32, tag="pconv")
                for off, nl, Pa, Pb in halves:
                    P, ci = (Pa, c) if c < 4 else (Pb, c - 4)
                    for k in range(K):
                        nc.tensor.matmul(
                            pc[:, off : off + nl],
                            identb_t,
                            P[:, k, ci, :nl],
                            start=(k == 0),
                            stop=(k == K - 1),
                        )
                st = st_pool.tile([128, SB], BF16, tag="stg")
                nc.scalar.copy(st[:, :nli], pc[:, :nli])
                nc.sync.dma_start(
                    out=out_d[:, c, CBL[i] : CBL[i + 1]], in_=st[:, :nli]
                )

        # ---- pipelined emission ----
        # front/mrep prep for block i+1 is emitted before conv(i) so
        # engines with in-order streams don't stall at block boundaries.
        def mark(label):
            PHASE_LOG.append((label, nc.next_id()))

        # Lookahead-2 pipeline: fronts/denoms/mrep for block i+2 are emitted
        # around conv(i), so the mrep PE->PSUM->ACT latency never gates the
        # conv muls, and DVE's recips run when their selsums are long done.
        load(0)
        load_rest_consts()
        for t in range(1, 8):
            load(t)
        mark("front0")
        front(0)
        denom(0)
        mark("mrep0")
        mrep(0)
        mark("front1")
        front(1)
        denom(1)
        mark("mrep1")
        mrep(1)
        t_done = 2
        for i in range(NI):
            mark(f"conv{i}m")
            conv_muls(i)
            mark(f"conv{i}t")
            conv_trees(i)
            if i + 2 < NI:
                new_ts = []
                while t_done < T_NEED[i + 2]:
                    mark(f"front{t_done}")
                    front(t_done)
                    new_ts.append(t_done)
                    t_done += 1
                mark(f"mrep{i + 2}")
                for t in new_ts:
                    denom(t)
                mrep(i + 2)
        mark("end")

    nc.compile()
    return nc


_CACHE = {}


def _get_program():
    if "nc" not in _CACHE:
        _CACHE["nc"] = build_program()
    return _CACHE["nc"]


def kernel(x, W, b):
    x = np.asarray(x, dtype=np.float32)
    assert x.shape == (B, S, D), x.shape

    nc = _get_program()
    consts = _host_constants(W, b)
    in_maps = []
    for core in range(B):
        in_maps.append(
            {
                "x": np.ascontiguousarray(x[core]),
                "consts": consts,
            }
        )
    res = bass_utils.run_bass_kernel_spmd(nc, in_maps, core_ids=list(range(B)))
    out = np.empty((B, L, D), dtype=np.float32)
    for core in range(B):
        ot = np.asarray(res.results[core]["out"])  # [128, C, L] bf16
        # d = 128*c + p  ->  [C, 128, L] -> [D, L] -> [L, D]
        out[core] = ot.transpose(1, 0, 2).reshape(D, L).T.astype(np.float32)
    return out
